# revision 2
# baseline (speedup 1.0000x reference)
"""MoE grouped-GEMM (8 experts) on 8 Trainium2 NeuronCores.

Problem: input [32768, 1024] routed contiguously to 8 experts (counts in
num_experts_per_token); expert i computes x_i @ W_i.T + b_i with
W [8, 4096, 1024], b [8, 4096]. Output [32768, 4096].

Sharding: expert-parallel, expert i <-> core i. Zero collectives: the host
slices each expert's token block, pre-transposes x and W so the contraction
dim (DIN) lands on SBUF partitions, each core runs a 4096x1024x4096 GEMM
(+bias), and the host concatenates the per-core outputs.

Device kernel (per core):
  - xT [1024, 4096]  (DIN x tokens)  resident in SBUF as 8 k-tiles [128, 4096]
  - wT [1024, 4096]  (DIN x DOUT)    streamed as 8 column blocks [1024, 512]
  - bias_b [128, 4096] host-broadcast bias, resident
  - y [4096, 4096] tokens x DOUT
  Matmuls use float32r (1 cycle/row at N=512 vs 4 for plain fp32; rel err
  ~1.5e-4 with fp32 PSUM accumulation).
"""

import sys

if "/opt/trn_rl_repo" not in sys.path:
    sys.path.insert(0, "/opt/trn_rl_repo")

import numpy as np

E, T, DIN, DOUT = 8, 32768, 1024, 4096
NCORES = 8
TOKC = T // NCORES  # tokens per core (capacity)

KT = 128   # contraction tile (SBUF partitions)
MT = 128   # token tile (PSUM partitions)
NT = 512   # dout tile (one fp32 PSUM bank)
KTILES = DIN // KT    # 8
MTILES = TOKC // MT   # 32
NTILES = DOUT // NT   # 8

_CACHE = {}


def _build_nc():
    import concourse.bacc as bacc
    import concourse.tile as tile
    import concourse.mybir as mybir

    nc = bacc.Bacc("TRN2", target_bir_lowering=False, debug=False,
                   num_devices=NCORES)

    xT = nc.dram_tensor("xT", [DIN, TOKC], mybir.dt.float32r,
                        kind="ExternalInput")
    wT = nc.dram_tensor("wT", [DIN, DOUT], mybir.dt.float32r,
                        kind="ExternalInput")
    bias_b = nc.dram_tensor("bias_b", [MT, DOUT], mybir.dt.float32,
                            kind="ExternalInput")
    y = nc.dram_tensor("y", [TOKC, DOUT], mybir.dt.float32,
                       kind="ExternalOutput")

    with tile.TileContext(nc) as tc:
        with (
            tc.tile_pool(name="xpool", bufs=1) as xpool,
            tc.tile_pool(name="wpool", bufs=2) as wpool,
            tc.tile_pool(name="bpool", bufs=1) as bpool,
            tc.tile_pool(name="opool", bufs=4) as opool,
            tc.tile_pool(name="psum", bufs=8, space="PSUM") as psum_pool,
        ):
            bias_t = bpool.tile([MT, DOUT], mybir.dt.float32,
                                name="bias_t", tag="bias_t")
            nc.sync.dma_start(bias_t[:], bias_b[:])

            # resident activations: 8 k-tiles of [128, TOKC]
            xt = []
            for k in range(KTILES):
                t = xpool.tile([KT, TOKC], mybir.dt.float32r,
                               name=f"xt{k}", tag=f"xt{k}")
                nc.sync.dma_start(t[:], xT[k * KT:(k + 1) * KT, :])
                xt.append(t)

            for n in range(NTILES):
                # stream one [DIN, NT] column block of wT (8 k-slices)
                wn = wpool.tile([KT, KTILES * NT], mybir.dt.float32r,
                                name="wn", tag="wn")
                for k in range(KTILES):
                    nc.sync.dma_start(
                        wn[:, k * NT:(k + 1) * NT],
                        wT[k * KT:(k + 1) * KT, n * NT:(n + 1) * NT])
                for m in range(MTILES):
                    acc = psum_pool.tile([MT, NT], mybir.dt.float32,
                                         name="acc", tag="acc")
                    for k in range(KTILES):
                        nc.tensor.matmul(
                            acc[:],
                            xt[k][:, m * MT:(m + 1) * MT],
                            wn[:, k * NT:(k + 1) * NT],
                            start=(k == 0), stop=(k == KTILES - 1))
                    ot = opool.tile([MT, NT], mybir.dt.float32,
                                    name="ot", tag="ot")
                    nc.vector.tensor_add(
                        ot[:], acc[:], bias_t[:, n * NT:(n + 1) * NT])
                    nc.scalar.dma_start(
                        y[m * MT:(m + 1) * MT, n * NT:(n + 1) * NT], ot[:])

    nc.compile()
    return nc


def _install_neff_cache():
    """Disk-cache walrus NEFF compiles keyed on the BIR bytes."""
    if _CACHE.get("neff_cache_installed"):
        return
    _CACHE["neff_cache_installed"] = True
    import hashlib
    import os
    import shutil

    import concourse.bass2jax as bass2jax

    cache_dir = "/root/.neff_bir_cache"
    os.makedirs(cache_dir, exist_ok=True)
    orig = bass2jax.compile_bir_kernel

    def cached_compile(ant_bir_str, tmpdir, neff_name="file.neff", **kw):
        key = hashlib.sha256(
            ant_bir_str if isinstance(ant_bir_str, bytes)
            else ant_bir_str.encode()).hexdigest()
        hit = os.path.join(cache_dir, key + ".neff")
        dst = os.path.join(tmpdir, neff_name)
        if os.path.exists(hit):
            shutil.copyfile(hit, dst)
            return dst
        out = orig(ant_bir_str, tmpdir, neff_name=neff_name, **kw)
        try:
            shutil.copyfile(out, hit)
        except OSError:
            pass
        return out

    bass2jax.compile_bir_kernel = cached_compile


def _get_nc():
    if "nc" not in _CACHE:
        _install_neff_cache()
        _CACHE["nc"] = _build_nc()
    return _CACHE["nc"]


def kernel(input, weight, bias, num_experts_per_token):
    from concourse.bass_utils import run_bass_kernel_spmd

    input = np.ascontiguousarray(np.asarray(input, dtype=np.float32))
    weight = np.ascontiguousarray(np.asarray(weight, dtype=np.float32))
    bias = np.ascontiguousarray(np.asarray(bias, dtype=np.float32))
    counts = np.asarray(num_experts_per_token).astype(np.int64)
    offsets = np.concatenate([[0], np.cumsum(counts)]).astype(np.int64)

    if counts.max() > TOKC:
        # capacity overflow (never hit with balanced routing): numpy fallback
        outs = []
        for i in range(E):
            xi = input[offsets[i]:offsets[i + 1]]
            outs.append(xi @ weight[i].T + bias[i])
        return np.concatenate(outs, axis=0)

    in_maps = []
    for i in range(E):
        xi = input[offsets[i]:offsets[i + 1]]  # [n_i, DIN]
        if xi.shape[0] < TOKC:
            xi = np.concatenate(
                [xi, np.zeros((TOKC - xi.shape[0], DIN), np.float32)], axis=0)
        xiT = np.ascontiguousarray(xi.T)                 # [DIN, TOKC]
        wiT = np.ascontiguousarray(weight[i].T)          # [DIN, DOUT]
        bb = np.ascontiguousarray(
            np.broadcast_to(bias[i][None, :], (MT, DOUT)))
        in_maps.append({"xT": xiT, "wT": wiT, "bias_b": bb})

    nc = _get_nc()
    import os
    trace = bool(int(os.environ.get("KERNEL_TRACE", "0")))
    res = run_bass_kernel_spmd(nc, in_maps, core_ids=list(range(NCORES)),
                               trace=trace)
    _CACHE["last_result"] = res

    out = np.empty((T, DOUT), dtype=np.float32)
    pos = 0
    for i in range(E):
        n_i = int(counts[i])
        out[pos:pos + n_i] = res.results[i]["y"][:n_i]
        pos += n_i
    return out


# revision 4
# speedup vs baseline: 1.0385x; 1.0385x over previous
"""MoE grouped-GEMM (8 experts) on 8 Trainium2 NeuronCores.

Problem: input [32768, 1024] routed contiguously to 8 experts (counts in
num_experts_per_token); expert i computes x_i @ W_i.T + b_i with
W [8, 4096, 1024], b [8, 4096]. Output [32768, 4096].

Sharding: expert-parallel, expert i <-> core i. Zero collectives: the host
slices each expert's token block, pre-transposes x and W so the contraction
dim (DIN) lands on SBUF partitions, each core runs a 4096x1024x4096 GEMM
(+bias), and the host concatenates the per-core outputs.

Device kernel (per core):
  - xT [1024, 4096]  (DIN x tokens)  resident in SBUF as 8 k-tiles [128, 4096]
  - wT [1024, 4096]  (DIN x DOUT)    streamed as 8 column blocks [1024, 512]
  - bias_b [128, 4096] host-broadcast bias, resident
  - y [4096, 4096] tokens x DOUT
  Matmuls use float32r (1 cycle/row at N=512 vs 4 for plain fp32; rel err
  ~1.5e-4 with fp32 PSUM accumulation).
"""

import sys

if "/opt/trn_rl_repo" not in sys.path:
    sys.path.insert(0, "/opt/trn_rl_repo")

import numpy as np

E, T, DIN, DOUT = 8, 32768, 1024, 4096
NCORES = 8
TOKC = T // NCORES  # tokens per core (capacity)

KT = 128   # contraction tile (SBUF partitions)
MT = 128   # token tile (PSUM partitions)
NT = 512   # dout tile (one fp32 PSUM bank)
KTILES = DIN // KT    # 8
MTILES = TOKC // MT   # 32
NTILES = DOUT // NT   # 8

_CACHE = {}


CT = 512                 # token chunk (streamed xT granularity)
CHUNKS = TOKC // CT      # 8
CMT = CT // MT           # 4 token tiles per chunk


def _build_nc():
    import concourse.bacc as bacc
    import concourse.tile as tile
    import concourse.mybir as mybir

    nc = bacc.Bacc("TRN2", target_bir_lowering=False, debug=False,
                   num_devices=NCORES)

    xT = nc.dram_tensor("xT", [DIN, TOKC], mybir.dt.float32r,
                        kind="ExternalInput")
    wT = nc.dram_tensor("wT", [DIN, DOUT], mybir.dt.float32r,
                        kind="ExternalInput")
    bias_b = nc.dram_tensor("bias_b", [MT, DOUT], mybir.dt.float32,
                            kind="ExternalInput")
    y = nc.dram_tensor("y", [TOKC, DOUT], mybir.dt.float32,
                       kind="ExternalOutput")

    with tile.TileContext(nc) as tc:
        with (
            tc.tile_pool(name="xpool", bufs=2) as xpool,
            tc.tile_pool(name="wpool", bufs=1) as wpool,
            tc.tile_pool(name="bpool", bufs=1) as bpool,
            tc.tile_pool(name="opool", bufs=6) as opool,
            tc.tile_pool(name="psum", bufs=8, space="PSUM") as psum_pool,
        ):
            # resident weights: 8 k-tiles of [128, DOUT]. The n=0 column
            # block is loaded first (small strided DMAs) so compute can
            # start early; the n=1..7 remainder follows as large
            # contiguous row DMAs that hide under chunk-0 compute.
            wt = [wpool.tile([KT, DOUT], mybir.dt.float32r,
                             name=f"wt{k}", tag=f"wt{k}")
                  for k in range(KTILES)]
            for k in range(KTILES):
                nc.sync.dma_start(wt[k][:, 0:NT],
                                  wT[k * KT:(k + 1) * KT, 0:NT])

            def load_chunk(c):
                tiles = []
                for k in range(KTILES):
                    t = xpool.tile([KT, CT], mybir.dt.float32r,
                                   name=f"xtc{k}", tag=f"xtc{k}")
                    nc.sync.dma_start(
                        t[:], xT[k * KT:(k + 1) * KT, c * CT:(c + 1) * CT])
                    tiles.append(t)
                return tiles

            xtc_cur = load_chunk(0)

            bias_t = bpool.tile([MT, DOUT], mybir.dt.float32,
                                name="bias_t", tag="bias_t")
            nc.sync.dma_start(bias_t[:], bias_b[:])

            # remainder of wT, n-major so column block n lands just before
            # chunk-0 compute reaches it
            for n in range(1, NTILES):
                for k in range(KTILES):
                    nc.sync.dma_start(
                        wt[k][:, n * NT:(n + 1) * NT],
                        wT[k * KT:(k + 1) * KT, n * NT:(n + 1) * NT])

            for c in range(CHUNKS):
                xtc_next = load_chunk(c + 1) if c + 1 < CHUNKS else None
                for n in range(NTILES):
                    for m in range(CMT):
                        acc = psum_pool.tile([MT, NT], mybir.dt.float32,
                                             name="acc", tag="acc")
                        for k in range(KTILES):
                            nc.tensor.matmul(
                                acc[:],
                                xtc_cur[k][:, m * MT:(m + 1) * MT],
                                wt[k][:, n * NT:(n + 1) * NT],
                                start=(k == 0), stop=(k == KTILES - 1))
                        ot = opool.tile([MT, NT], mybir.dt.float32,
                                        name="ot", tag="ot")
                        nc.vector.tensor_add(
                            ot[:], acc[:], bias_t[:, n * NT:(n + 1) * NT])
                        row0 = c * CT + m * MT
                        nc.scalar.dma_start(
                            y[row0:row0 + MT, n * NT:(n + 1) * NT], ot[:])
                xtc_cur = xtc_next

    nc.compile()
    return nc


def _install_neff_cache():
    """Disk-cache walrus NEFF compiles keyed on the BIR bytes."""
    if _CACHE.get("neff_cache_installed"):
        return
    _CACHE["neff_cache_installed"] = True
    import hashlib
    import os
    import shutil

    import concourse.bass2jax as bass2jax

    cache_dir = "/root/.neff_bir_cache"
    os.makedirs(cache_dir, exist_ok=True)
    orig = bass2jax.compile_bir_kernel

    def cached_compile(ant_bir_str, tmpdir, neff_name="file.neff", **kw):
        key = hashlib.sha256(
            ant_bir_str if isinstance(ant_bir_str, bytes)
            else ant_bir_str.encode()).hexdigest()
        hit = os.path.join(cache_dir, key + ".neff")
        dst = os.path.join(tmpdir, neff_name)
        if os.path.exists(hit):
            shutil.copyfile(hit, dst)
            return dst
        out = orig(ant_bir_str, tmpdir, neff_name=neff_name, **kw)
        try:
            shutil.copyfile(out, hit)
        except OSError:
            pass
        return out

    bass2jax.compile_bir_kernel = cached_compile


def _get_nc():
    if "nc" not in _CACHE:
        _install_neff_cache()
        _CACHE["nc"] = _build_nc()
    return _CACHE["nc"]


def kernel(input, weight, bias, num_experts_per_token):
    from concourse.bass_utils import run_bass_kernel_spmd

    input = np.ascontiguousarray(np.asarray(input, dtype=np.float32))
    weight = np.ascontiguousarray(np.asarray(weight, dtype=np.float32))
    bias = np.ascontiguousarray(np.asarray(bias, dtype=np.float32))
    counts = np.asarray(num_experts_per_token).astype(np.int64)
    offsets = np.concatenate([[0], np.cumsum(counts)]).astype(np.int64)

    if counts.max() > TOKC:
        # capacity overflow (never hit with balanced routing): numpy fallback
        outs = []
        for i in range(E):
            xi = input[offsets[i]:offsets[i + 1]]
            outs.append(xi @ weight[i].T + bias[i])
        return np.concatenate(outs, axis=0)

    in_maps = []
    for i in range(E):
        xi = input[offsets[i]:offsets[i + 1]]  # [n_i, DIN]
        if xi.shape[0] < TOKC:
            xi = np.concatenate(
                [xi, np.zeros((TOKC - xi.shape[0], DIN), np.float32)], axis=0)
        xiT = np.ascontiguousarray(xi.T)                 # [DIN, TOKC]
        wiT = np.ascontiguousarray(weight[i].T)          # [DIN, DOUT]
        bb = np.ascontiguousarray(
            np.broadcast_to(bias[i][None, :], (MT, DOUT)))
        in_maps.append({"xT": xiT, "wT": wiT, "bias_b": bb})

    nc = _get_nc()
    import os
    trace = bool(int(os.environ.get("KERNEL_TRACE", "0")))
    res = run_bass_kernel_spmd(nc, in_maps, core_ids=list(range(NCORES)),
                               trace=trace)
    _CACHE["last_result"] = res

    out = np.empty((T, DOUT), dtype=np.float32)
    pos = 0
    for i in range(E):
        n_i = int(counts[i])
        out[pos:pos + n_i] = res.results[i]["y"][:n_i]
        pos += n_i
    return out


# revision 5
# speedup vs baseline: 1.0407x; 1.0022x over previous
"""MoE grouped-GEMM (8 experts) on 8 Trainium2 NeuronCores.

Problem: input [32768, 1024] routed contiguously to 8 experts (counts in
num_experts_per_token); expert i computes x_i @ W_i.T + b_i with
W [8, 4096, 1024], b [8, 4096]. Output [32768, 4096].

Sharding: expert-parallel, expert i <-> core i. Zero collectives: the host
slices each expert's token block, pre-transposes x and W so the contraction
dim (DIN) lands on SBUF partitions, each core runs a 4096x1024x4096 GEMM
(+bias), and the host concatenates the per-core outputs.

Device kernel (per core):
  - xT [1024, 4096]  (DIN x tokens)  resident in SBUF as 8 k-tiles [128, 4096]
  - wT [1024, 4096]  (DIN x DOUT)    streamed as 8 column blocks [1024, 512]
  - bias_b [128, 4096] host-broadcast bias, resident
  - y [4096, 4096] tokens x DOUT
  Matmuls use float32r (1 cycle/row at N=512 vs 4 for plain fp32; rel err
  ~1.5e-4 with fp32 PSUM accumulation).
"""

import sys

if "/opt/trn_rl_repo" not in sys.path:
    sys.path.insert(0, "/opt/trn_rl_repo")

import numpy as np

E, T, DIN, DOUT = 8, 32768, 1024, 4096
NCORES = 8
TOKC = T // NCORES  # tokens per core (capacity)

KT = 128   # contraction tile (SBUF partitions)
MT = 128   # token tile (PSUM partitions)
NT = 512   # dout tile (one fp32 PSUM bank)
KTILES = DIN // KT    # 8
MTILES = TOKC // MT   # 32
NTILES = DOUT // NT   # 8

_CACHE = {}


CT = 512                 # token chunk (streamed xT granularity)
CHUNKS = TOKC // CT      # 8
CMT = CT // MT           # 4 token tiles per chunk


def _build_nc():
    import concourse.bacc as bacc
    import concourse.tile as tile
    import concourse.mybir as mybir

    nc = bacc.Bacc("TRN2", target_bir_lowering=False, debug=False,
                   num_devices=NCORES)

    xT = nc.dram_tensor("xT", [DIN, TOKC], mybir.dt.float32r,
                        kind="ExternalInput")
    wT = nc.dram_tensor("wT", [DIN, DOUT], mybir.dt.float32r,
                        kind="ExternalInput")
    bias_b = nc.dram_tensor("bias_b", [MT, DOUT], mybir.dt.float32,
                            kind="ExternalInput")
    y = nc.dram_tensor("y", [TOKC, DOUT], mybir.dt.float32,
                       kind="ExternalOutput")

    with tile.TileContext(nc) as tc:
        with (
            tc.tile_pool(name="xpool", bufs=2) as xpool,
            tc.tile_pool(name="wpool", bufs=1) as wpool,
            tc.tile_pool(name="bpool", bufs=1) as bpool,
            tc.tile_pool(name="opool", bufs=6) as opool,
            tc.tile_pool(name="psum", bufs=8, space="PSUM") as psum_pool,
        ):
            # resident weights: 8 k-tiles of [128, DOUT]. The n=0 column
            # block is loaded first (small strided DMAs) so compute can
            # start early; the n=1..7 remainder follows as large
            # contiguous row DMAs that hide under chunk-0 compute.
            wt = [wpool.tile([KT, DOUT], mybir.dt.float32r,
                             name=f"wt{k}", tag=f"wt{k}")
                  for k in range(KTILES)]
            for k in range(KTILES):
                nc.sync.dma_start(wt[k][:, 0:NT],
                                  wT[k * KT:(k + 1) * KT, 0:NT])

            def load_chunk(c):
                # scalar (ACT) HWDGE ring: runs in parallel with the weight
                # loads on the sync (SP) ring during the input burst
                tiles = []
                for k in range(KTILES):
                    t = xpool.tile([KT, CT], mybir.dt.float32r,
                                   name=f"xtc{k}", tag=f"xtc{k}")
                    nc.scalar.dma_start(
                        t[:], xT[k * KT:(k + 1) * KT, c * CT:(c + 1) * CT])
                    tiles.append(t)
                return tiles

            xtc_cur = load_chunk(0)

            # remainder of wT, n-major so column block n lands just before
            # chunk-0 compute reaches it; bias (needed ~20us in) rides
            # between n=1 and n=2
            bias_t = bpool.tile([MT, DOUT], mybir.dt.float32,
                                name="bias_t", tag="bias_t")
            for n in range(1, NTILES):
                for k in range(KTILES):
                    nc.sync.dma_start(
                        wt[k][:, n * NT:(n + 1) * NT],
                        wT[k * KT:(k + 1) * KT, n * NT:(n + 1) * NT])
                if n == 1:
                    nc.sync.dma_start(bias_t[:], bias_b[:])

            for c in range(CHUNKS):
                xtc_next = load_chunk(c + 1) if c + 1 < CHUNKS else None
                for n in range(NTILES):
                    for m in range(CMT):
                        acc = psum_pool.tile([MT, NT], mybir.dt.float32,
                                             name="acc", tag="acc")
                        for k in range(KTILES):
                            nc.tensor.matmul(
                                acc[:],
                                xtc_cur[k][:, m * MT:(m + 1) * MT],
                                wt[k][:, n * NT:(n + 1) * NT],
                                start=(k == 0), stop=(k == KTILES - 1))
                        ot = opool.tile([MT, NT], mybir.dt.float32,
                                        name="ot", tag="ot")
                        nc.vector.tensor_add(
                            ot[:], acc[:], bias_t[:, n * NT:(n + 1) * NT])
                        row0 = c * CT + m * MT
                        nc.scalar.dma_start(
                            y[row0:row0 + MT, n * NT:(n + 1) * NT], ot[:])
                xtc_cur = xtc_next

    nc.compile()
    return nc


def _install_neff_cache():
    """Disk-cache walrus NEFF compiles keyed on the BIR bytes."""
    if _CACHE.get("neff_cache_installed"):
        return
    _CACHE["neff_cache_installed"] = True
    import hashlib
    import os
    import shutil

    import concourse.bass2jax as bass2jax

    cache_dir = "/root/.neff_bir_cache"
    os.makedirs(cache_dir, exist_ok=True)
    orig = bass2jax.compile_bir_kernel

    def cached_compile(ant_bir_str, tmpdir, neff_name="file.neff", **kw):
        key = hashlib.sha256(
            ant_bir_str if isinstance(ant_bir_str, bytes)
            else ant_bir_str.encode()).hexdigest()
        hit = os.path.join(cache_dir, key + ".neff")
        dst = os.path.join(tmpdir, neff_name)
        if os.path.exists(hit):
            shutil.copyfile(hit, dst)
            return dst
        out = orig(ant_bir_str, tmpdir, neff_name=neff_name, **kw)
        try:
            shutil.copyfile(out, hit)
        except OSError:
            pass
        return out

    bass2jax.compile_bir_kernel = cached_compile


def _get_nc():
    if "nc" not in _CACHE:
        _install_neff_cache()
        _CACHE["nc"] = _build_nc()
    return _CACHE["nc"]


def kernel(input, weight, bias, num_experts_per_token):
    from concourse.bass_utils import run_bass_kernel_spmd

    input = np.ascontiguousarray(np.asarray(input, dtype=np.float32))
    weight = np.ascontiguousarray(np.asarray(weight, dtype=np.float32))
    bias = np.ascontiguousarray(np.asarray(bias, dtype=np.float32))
    counts = np.asarray(num_experts_per_token).astype(np.int64)
    offsets = np.concatenate([[0], np.cumsum(counts)]).astype(np.int64)

    if counts.max() > TOKC:
        # capacity overflow (never hit with balanced routing): numpy fallback
        outs = []
        for i in range(E):
            xi = input[offsets[i]:offsets[i + 1]]
            outs.append(xi @ weight[i].T + bias[i])
        return np.concatenate(outs, axis=0)

    in_maps = []
    for i in range(E):
        xi = input[offsets[i]:offsets[i + 1]]  # [n_i, DIN]
        if xi.shape[0] < TOKC:
            xi = np.concatenate(
                [xi, np.zeros((TOKC - xi.shape[0], DIN), np.float32)], axis=0)
        xiT = np.ascontiguousarray(xi.T)                 # [DIN, TOKC]
        wiT = np.ascontiguousarray(weight[i].T)          # [DIN, DOUT]
        bb = np.ascontiguousarray(
            np.broadcast_to(bias[i][None, :], (MT, DOUT)))
        in_maps.append({"xT": xiT, "wT": wiT, "bias_b": bb})

    nc = _get_nc()
    import os
    trace = bool(int(os.environ.get("KERNEL_TRACE", "0")))
    res = run_bass_kernel_spmd(nc, in_maps, core_ids=list(range(NCORES)),
                               trace=trace)
    _CACHE["last_result"] = res

    out = np.empty((T, DOUT), dtype=np.float32)
    pos = 0
    for i in range(E):
        n_i = int(counts[i])
        out[pos:pos + n_i] = res.results[i]["y"][:n_i]
        pos += n_i
    return out


# revision 6
# speedup vs baseline: 1.0714x; 1.0295x over previous
"""MoE grouped-GEMM (8 experts) on 8 Trainium2 NeuronCores.

Problem: input [32768, 1024] routed contiguously to 8 experts (counts in
num_experts_per_token); expert i computes x_i @ W_i.T + b_i with
W [8, 4096, 1024], b [8, 4096]. Output [32768, 4096].

Sharding: expert-parallel, expert i <-> core i. Zero collectives: the host
slices each expert's token block, pre-transposes x and W so the contraction
dim (DIN) lands on SBUF partitions, each core runs a 4096x1024x4096 GEMM
(+bias), and the host concatenates the per-core outputs.

Device kernel (per core):
  - xT [1024, 4096]  (DIN x tokens)  resident in SBUF as 8 k-tiles [128, 4096]
  - wT [1024, 4096]  (DIN x DOUT)    streamed as 8 column blocks [1024, 512]
  - bias_b [128, 4096] host-broadcast bias, resident
  - y [4096, 4096] tokens x DOUT
  Matmuls use float32r (1 cycle/row at N=512 vs 4 for plain fp32; rel err
  ~1.5e-4 with fp32 PSUM accumulation).
"""

import sys

if "/opt/trn_rl_repo" not in sys.path:
    sys.path.insert(0, "/opt/trn_rl_repo")

import numpy as np

E, T, DIN, DOUT = 8, 32768, 1024, 4096
NCORES = 8
TOKC = T // NCORES  # tokens per core (capacity)

KT = 128   # contraction tile (SBUF partitions)
MT = 128   # token tile (PSUM partitions)
NT = 512   # dout tile (one fp32 PSUM bank)
KTILES = DIN // KT    # 8
MTILES = TOKC // MT   # 32
NTILES = DOUT // NT   # 8

_CACHE = {}


CT = 512                 # token chunk (streamed xT granularity)
CHUNKS = TOKC // CT      # 8
CMT = CT // MT           # 4 token tiles per chunk


def _build_nc():
    import concourse.bacc as bacc
    import concourse.tile as tile
    import concourse.mybir as mybir

    nc = bacc.Bacc("TRN2", target_bir_lowering=False, debug=False,
                   num_devices=NCORES)

    xT = nc.dram_tensor("xT", [DIN, TOKC], mybir.dt.float32r,
                        kind="ExternalInput")
    wT = nc.dram_tensor("wT", [DIN, DOUT], mybir.dt.float32r,
                        kind="ExternalInput")
    bias_b = nc.dram_tensor("bias_b", [MT, DOUT], mybir.dt.float32,
                            kind="ExternalInput")
    y = nc.dram_tensor("y", [TOKC, DOUT], mybir.dt.float32,
                       kind="ExternalOutput")

    with tile.TileContext(nc) as tc:
        with (
            tc.tile_pool(name="xpool", bufs=2) as xpool,
            tc.tile_pool(name="wpool", bufs=1) as wpool,
            tc.tile_pool(name="bpool", bufs=1) as bpool,
            tc.tile_pool(name="opool", bufs=6) as opool,
            tc.tile_pool(name="psum", bufs=8, space="PSUM") as psum_pool,
        ):
            # resident weights: 8 k-tiles of [128, DOUT]. The n=0 column
            # block is loaded first (small strided DMAs) so compute can
            # start early; the n=1..7 remainder follows as large
            # contiguous row DMAs that hide under chunk-0 compute.
            wt = [wpool.tile([KT, DOUT], mybir.dt.float32r,
                             name=f"wt{k}", tag=f"wt{k}")
                  for k in range(KTILES)]
            for k in range(KTILES):
                nc.sync.dma_start(wt[k][:, 0:NT],
                                  wT[k * KT:(k + 1) * KT, 0:NT])

            def load_chunk(c):
                # scalar (ACT) HWDGE ring: runs in parallel with the weight
                # loads on the sync (SP) ring during the input burst
                tiles = []
                for k in range(KTILES):
                    t = xpool.tile([KT, CT], mybir.dt.float32r,
                                   name=f"xtc{k}", tag=f"xtc{k}")
                    nc.scalar.dma_start(
                        t[:], xT[k * KT:(k + 1) * KT, c * CT:(c + 1) * CT])
                    tiles.append(t)
                return tiles

            xtc_cur = load_chunk(0)

            # wT n=1..3 blocks, n-major so block n lands just before
            # phase-0 chunk-0 compute reaches it; bias (needed ~20us in)
            # rides between n=1 and n=2. n=4..7 load mid-phase-0 (needed
            # only by phase 1), keeping chunk-0's HBM read demand under
            # the per-core cap.
            NPH = NTILES // 2  # n-blocks per phase
            bias_t = bpool.tile([MT, DOUT], mybir.dt.float32,
                                name="bias_t", tag="bias_t")
            for n in range(1, NPH):
                for k in range(KTILES):
                    nc.sync.dma_start(
                        wt[k][:, n * NT:(n + 1) * NT],
                        wT[k * KT:(k + 1) * KT, n * NT:(n + 1) * NT])
                if n == 1:
                    nc.sync.dma_start(bias_t[:], bias_b[:])

            for phase in range(2):
                for c in range(CHUNKS):
                    if phase == 0 and c == 3:
                        # phase-1 weight columns; plenty of ring headroom now
                        for n in range(NPH, NTILES):
                            for k in range(KTILES):
                                nc.sync.dma_start(
                                    wt[k][:, n * NT:(n + 1) * NT],
                                    wT[k * KT:(k + 1) * KT,
                                       n * NT:(n + 1) * NT])
                    last = phase == 1 and c == CHUNKS - 1
                    xtc_next = None if last else load_chunk(
                        (c + 1) % CHUNKS)
                    for n in range(phase * NPH, (phase + 1) * NPH):
                        for m in range(CMT):
                            acc = psum_pool.tile([MT, NT], mybir.dt.float32,
                                                 name="acc", tag="acc")
                            for k in range(KTILES):
                                nc.tensor.matmul(
                                    acc[:],
                                    xtc_cur[k][:, m * MT:(m + 1) * MT],
                                    wt[k][:, n * NT:(n + 1) * NT],
                                    start=(k == 0), stop=(k == KTILES - 1))
                            ot = opool.tile([MT, NT], mybir.dt.float32,
                                            name="ot", tag="ot")
                            nc.vector.tensor_add(
                                ot[:], acc[:], bias_t[:, n * NT:(n + 1) * NT])
                            row0 = c * CT + m * MT
                            nc.scalar.dma_start(
                                y[row0:row0 + MT, n * NT:(n + 1) * NT], ot[:])
                    xtc_cur = xtc_next

    nc.compile()
    return nc


def _install_neff_cache():
    """Disk-cache walrus NEFF compiles keyed on the BIR bytes."""
    if _CACHE.get("neff_cache_installed"):
        return
    _CACHE["neff_cache_installed"] = True
    import hashlib
    import os
    import shutil

    import concourse.bass2jax as bass2jax

    cache_dir = "/root/.neff_bir_cache"
    os.makedirs(cache_dir, exist_ok=True)
    orig = bass2jax.compile_bir_kernel

    def cached_compile(ant_bir_str, tmpdir, neff_name="file.neff", **kw):
        key = hashlib.sha256(
            ant_bir_str if isinstance(ant_bir_str, bytes)
            else ant_bir_str.encode()).hexdigest()
        hit = os.path.join(cache_dir, key + ".neff")
        dst = os.path.join(tmpdir, neff_name)
        if os.path.exists(hit):
            shutil.copyfile(hit, dst)
            return dst
        out = orig(ant_bir_str, tmpdir, neff_name=neff_name, **kw)
        try:
            shutil.copyfile(out, hit)
        except OSError:
            pass
        return out

    bass2jax.compile_bir_kernel = cached_compile


def _get_nc():
    if "nc" not in _CACHE:
        _install_neff_cache()
        _CACHE["nc"] = _build_nc()
    return _CACHE["nc"]


def kernel(input, weight, bias, num_experts_per_token):
    from concourse.bass_utils import run_bass_kernel_spmd

    input = np.ascontiguousarray(np.asarray(input, dtype=np.float32))
    weight = np.ascontiguousarray(np.asarray(weight, dtype=np.float32))
    bias = np.ascontiguousarray(np.asarray(bias, dtype=np.float32))
    counts = np.asarray(num_experts_per_token).astype(np.int64)
    offsets = np.concatenate([[0], np.cumsum(counts)]).astype(np.int64)

    if counts.max() > TOKC:
        # capacity overflow (never hit with balanced routing): numpy fallback
        outs = []
        for i in range(E):
            xi = input[offsets[i]:offsets[i + 1]]
            outs.append(xi @ weight[i].T + bias[i])
        return np.concatenate(outs, axis=0)

    in_maps = []
    for i in range(E):
        xi = input[offsets[i]:offsets[i + 1]]  # [n_i, DIN]
        if xi.shape[0] < TOKC:
            xi = np.concatenate(
                [xi, np.zeros((TOKC - xi.shape[0], DIN), np.float32)], axis=0)
        xiT = np.ascontiguousarray(xi.T)                 # [DIN, TOKC]
        wiT = np.ascontiguousarray(weight[i].T)          # [DIN, DOUT]
        bb = np.ascontiguousarray(
            np.broadcast_to(bias[i][None, :], (MT, DOUT)))
        in_maps.append({"xT": xiT, "wT": wiT, "bias_b": bb})

    nc = _get_nc()
    import os
    trace = bool(int(os.environ.get("KERNEL_TRACE", "0")))
    res = run_bass_kernel_spmd(nc, in_maps, core_ids=list(range(NCORES)),
                               trace=trace)
    _CACHE["last_result"] = res

    out = np.empty((T, DOUT), dtype=np.float32)
    pos = 0
    for i in range(E):
        n_i = int(counts[i])
        out[pos:pos + n_i] = res.results[i]["y"][:n_i]
        pos += n_i
    return out
